# revision 1
# baseline (speedup 1.0000x reference)
"""Trainium2 Bass kernel v3 for nn_PointWiseMLP (ball query + gather + MLP + pool).

Self-contained: kernel(**inputs) shards across 8 NeuronCores and returns the
full [2, 128, 8192] output.

v3 vs v2:
- queries sorted by (x-slice, y) per batch; each 128-query tile gets its own
  compacted support window (<= 768 incl. slot 0 = original support 0), so the
  d2 matmul + selection run on 768 candidates instead of 2560.
- per-tile G/V gather tables [128, 1536] built by f32r matmuls from per-tile
  gvrhs columns; per-tile ap_gather (cost ~ max(1536, 1056)).
- unclamped rank scans (ranks distinct -> no scatter duplicates) drop the
  gate ops; scatter outputs sized to the max rank.
- bf16 h3 + tensor_tensor tree pooling (2x DVE mode) instead of 4 reduces.
- block-batched fill ops.
"""
import sys
for _p in ("/opt/trn_rl_repo", "/root/.axon_site/_ro/trn_rl_repo"):
    if _p not in sys.path:
        sys.path.append(_p)

import numpy as np
from contextlib import ExitStack

import concourse.bass as bass
import concourse.tile as tile
from concourse import mybir
from concourse._compat import with_exitstack

F32 = mybir.dt.float32
F32R = mybir.dt.float32r
BF16 = mybir.dt.bfloat16
I16 = mybir.dt.int16

RADIUS = 0.1
NSAMPLE = 32
EPS = 1e-5
NQ = 2048          # queries per core
NQT = 16           # query tiles per core
NT2 = 704          # support-window slots per tile (measured max 677)
NTC = NQT * NT2    # 12288 total window cols per core
NG = NT2 // 8      # 96 8-groups
NSLOT = 32
SEL2 = NSLOT * 8   # 256
GBLK = 4
NBLK = NQT // GBLK
NIDX = 1024 + 32   # gather indices per tile (G + V-center)
BIG = 1000.0
R2 = float(np.float32(0.01))

ALU = mybir.AluOpType
ACTF = mybir.ActivationFunctionType

CHUNKS = [(0, 512), (512, 192)]

# packed f32 consts: mh0[0:128] mh1[128:256] ident[256:384] qfm[384:400]
# t0v4[400] t1v4[401] t2v[402]
# packed i16 consts: iotac[0:NG] shv[NG:NG+8] tpat16[NG+8:NG+8+SEL2]
IN_SPECS = dict(
    lhsq=((13, NQ), F32), rhsd2=((13, NTC), F32),
    gvrhs=((67, NTC), F32R),
    qt3=((3, NQ), BF16), p0rT=((3, 128), BF16),
    gvl=((67, 256), F32R),
    w1blk=((128, 128), F32R), w2t4=((128, 128), F32R),
    cf32=((128, 403), F32),
    ci16=((128, NG + 8 + SEL2), I16),
    onesk1=((1, 128), F32),
    pow8=((128, 512), BF16),
)


def _split_hilo(x):
    x = x.astype(np.float32)
    hi = np.floor(x.astype(np.float64) * 1024.0) / 1024.0
    hi = hi.astype(np.float32)
    lo = (x - hi).astype(np.float32)
    return hi, lo


def host_prep(inputs):
    import ml_dtypes
    B = 2
    qx = np.asarray(inputs['query_xyz'], np.float32)
    sx = np.asarray(inputs['support_xyz'], np.float32)
    qm = np.asarray(inputs['query_mask'], np.int32)
    sm = np.asarray(inputs['support_mask'], np.int32)
    sf = np.asarray(inputs['support_features'], np.float32)

    W0 = np.asarray(inputs['W0'], np.float64)
    W1 = np.asarray(inputs['W1'], np.float64)
    W2 = np.asarray(inputs['W2'], np.float64)

    def fold(Wl, g, b, rm, rv):
        s = np.asarray(g, np.float64) / np.sqrt(np.asarray(rv, np.float64) + EPS)
        return Wl * s[:, None], np.asarray(b, np.float64) - np.asarray(rm, np.float64) * s

    W0p, t0 = fold(W0, inputs['g0'], inputs['b0'], inputs['rm0'], inputs['rv0'])
    W1p, t1 = fold(W1, inputs['g1'], inputs['b1'], inputs['rm1'], inputs['rv1'])
    W2p, t2 = fold(W2, inputs['g2'], inputs['b2'], inputs['rm2'], inputs['rv2'])

    P0 = (W0p[:, 0:3] / RADIUS).astype(np.float32)
    C0 = W0p[:, 3:67].astype(np.float32)
    D0 = W0p[:, 67:131].astype(np.float32)

    glhs = np.zeros((67, 128), np.float32)
    vlhs = np.zeros((67, 128), np.float32)
    for p in range(128):
        c = p % 32
        glhs[0:64, p] = D0[c]
        glhs[64:67, p] = P0[c]
        vlhs[0:64, p] = (C0 - D0)[c]

    p0rT = np.tile(-P0.T, (1, 4)).astype(np.float32)
    w1blk = np.zeros((128, 128), np.float32)
    W1f = W1p.astype(np.float32)
    for u in range(4):
        w1blk[32 * u:32 * u + 32, 32 * u:32 * u + 32] = W1f.T
    w2t4 = np.tile(W2p.T.astype(np.float32), (4, 1))

    t0v4 = np.tile(t0.astype(np.float32), 4).reshape(128, 1)
    t1v4 = np.tile(t1.astype(np.float32), 4).reshape(128, 1)
    t2v = t2.astype(np.float32).reshape(128, 1)

    Mh = np.zeros((2, 128, 128), np.float32)
    for h in range(2):
        for p in range(128):
            Mh[h, 32 * (p // 32) + 16 * h + p % 16, p] = 1.0
    ident = np.eye(128, dtype=np.float32)
    onesk1 = np.ones((1, 128), np.float32)

    pow8 = np.tile((2.0 ** (np.arange(512) % 8)).astype(np.float32)[None, :], (128, 1))
    iotac = np.tile((np.arange(NG, dtype=np.int16) + 1)[None, :], (128, 1))
    shv = np.tile(np.arange(8, dtype=np.int16)[None, :], (128, 1))
    tpat16 = np.tile((np.tile(np.arange(8, dtype=np.int16), NSLOT) - 7)[None, :],
                     (128, 1))

    npdt = {F32: np.float32, F32R: np.float32, BF16: ml_dtypes.bfloat16,
            I16: np.int16}
    in_maps = []
    meta = []
    for core in range(8):
        b = core // 4
        sl = core % 4
        xorder = np.argsort(qx[b][:, 0], kind='stable')
        xsel = xorder[sl * NQ:(sl + 1) * NQ]
        # within the x-slice, sort by y so each 128-tile is a y-band
        yorder = np.argsort(qx[b][xsel][:, 1], kind='stable')
        qsel = xsel[yorder]
        q = qx[b][qsel]
        qmk = qm[b][qsel].astype(np.float32)

        rhsd2 = np.zeros((13, NTC), np.float32)
        gvrhs = np.zeros((67, NTC), np.float32)
        rhsd2[3, :] = 1.0
        rhsd2[11, :] = 1.0
        rhsd2[12, :] = BIG
        for i in range(NQT):
            qt = q[128 * i:128 * (i + 1)]
            x0, x1 = qt[:, 0].min() - RADIUS, qt[:, 0].max() + RADIUS
            y0, y1 = qt[:, 1].min() - RADIUS, qt[:, 1].max() + RADIUS
            cand = ((sx[b][:, 0] >= x0) & (sx[b][:, 0] <= x1)
                    & (sx[b][:, 1] >= y0) & (sx[b][:, 1] <= y1) & (sm[b] > 0))
            cand[0] = False
            sids = np.concatenate([[0], np.flatnonzero(cand)]).astype(np.int64)
            n_w = len(sids)
            assert n_w <= NT2, f"window overflow: {n_w} > {NT2}"
            s_w = sx[b][sids]
            svalid = np.ones(n_w, np.float32)
            svalid[0] = float(sm[b][0] > 0)

            sh, slo = _split_hilo(s_w)
            s64, sh64 = s_w.astype(np.float64), sh.astype(np.float64)
            Ls = (np.sum(s64 * s64, 1) - np.sum(sh64 * sh64, 1)).astype(np.float32)
            o = NT2 * i
            rhsd2[0:3, o:o + n_w] = sh.T
            rhsd2[4, o:o + n_w] = np.sum(sh * sh, 1, dtype=np.float64).astype(np.float32)
            rhsd2[5:8, o:o + n_w] = -2.0 * sh.T
            rhsd2[8:11, o:o + n_w] = -2.0 * slo.T
            rhsd2[12, o:o + n_w] = Ls + BIG * (1.0 - svalid)
            gvrhs[0:64, o:o + n_w] = sf[b][:, sids]
            gvrhs[64:67, o:o + n_w] = s_w.T

        qh, ql = _split_hilo(q)
        q64, qh64 = q.astype(np.float64), qh.astype(np.float64)
        Lq = (np.sum(q64 * q64, 1) - np.sum(qh64 * qh64, 1)).astype(np.float32)
        lhsq = np.zeros((13, NQ), np.float32)
        lhsq[0:3] = -2.0 * qh.T
        lhsq[3] = np.sum(qh * qh, 1, dtype=np.float64).astype(np.float32)
        lhsq[4] = 1.0
        lhsq[5:8] = ql.T
        lhsq[8:11] = q.T
        lhsq[11] = Lq + BIG * (1.0 - qmk)
        lhsq[12] = 1.0

        qfm = qmk.reshape(NQT, 128).T
        cf32 = np.concatenate(
            [Mh[0], Mh[1], ident, qfm, t0v4, t1v4, t2v], axis=1)
        ci16 = np.concatenate([iotac, shv, tpat16], axis=1)
        im = dict(
            lhsq=lhsq, rhsd2=rhsd2, gvrhs=gvrhs,
            qt3=q.T.copy(), p0rT=p0rT,
            gvl=np.concatenate([glhs, vlhs], axis=1),
            w1blk=w1blk, w2t4=w2t4, cf32=cf32, ci16=ci16,
            onesk1=onesk1, pow8=pow8,
        )
        for k in im:
            shape, dt = IN_SPECS[k]
            arr = np.ascontiguousarray(im[k]).astype(npdt[dt])
            assert arr.shape == shape, (k, arr.shape, shape)
            im[k] = arr
        in_maps.append(im)
        meta.append((b, qsel))
    return in_maps, meta


def host_finish(results, meta):
    out = np.zeros((2, 128, 8192), np.float32)
    for core in range(8):
        b, qsel = meta[core]
        out[b][:, qsel] = results[core]['out']
    return out


# --------------------------------------------------------------------------
# device kernel
# --------------------------------------------------------------------------

@with_exitstack
def build_kernel(ctx: ExitStack, tc: tile.TileContext, out_ap: bass.AP, ins: dict):
    nc = tc.nc
    ctx.enter_context(nc.allow_low_precision("f32r mlp + small-int selection"))

    consts = ctx.enter_context(tc.tile_pool(name="consts", bufs=1))
    gvp = ctx.enter_context(tc.tile_pool(name="gv", bufs=1))
    tabp = ctx.enter_context(tc.tile_pool(name="tabp", bufs=5))
    selbig = ctx.enter_context(tc.tile_pool(name="selbig", bufs=2))
    sel = ctx.enter_context(tc.tile_pool(name="sel", bufs=2))
    small = ctx.enter_context(tc.tile_pool(name="small", bufs=2))
    blkp = ctx.enter_context(tc.tile_pool(name="blkp", bufs=2))
    idxp = ctx.enter_context(tc.tile_pool(name="idxp", bufs=3))
    gpool = ctx.enter_context(tc.tile_pool(name="gpool", bufs=4))
    mlpp = ctx.enter_context(tc.tile_pool(name="mlpp", bufs=2))
    h3p = ctx.enter_context(tc.tile_pool(name="h3p", bufs=2))
    outp = ctx.enter_context(tc.tile_pool(name="outb", bufs=1))
    psA = ctx.enter_context(tc.tile_pool(name="psA", bufs=2, space="PSUM"))
    psL2 = ctx.enter_context(tc.tile_pool(name="psL2", bufs=1, space="PSUM"))
    psL3 = ctx.enter_context(tc.tile_pool(name="psL3", bufs=2, space="PSUM"))

    QTR = 4 * NT2            # window cols per quarter (4 tiles)
    # window tensors: two quarter buffers, loaded a block ahead
    rhsTq = [gvp.tile([13, QTR], F32, tag=f"rhsT{j}", name=f"rhsT{j}") for j in range(2)]
    gvrTq = [gvp.tile([67, QTR], F32R, tag=f"gvrT{j}", name=f"gvrT{j}") for j in range(2)]

    def load_quarter(g, split=1):
        buf = g % 2
        E = QTR // split
        for e in range(split):
            o = g * QTR + e * E
            lo = e * E
            nc.sync.dma_start(out=gvrTq[buf][:, lo:lo + E],
                              in_=ins['gvrhs'][:, o:o + E])
            nc.sync.dma_start(out=rhsTq[buf][:, lo:lo + E],
                              in_=ins['rhsd2'][:, o:o + E])

    ct = {}
    # DMA order: the ops of tile 0 need gvl/lhsq/pow8/ci16 + first window
    # quarter; everything else trickles in behind.
    first = ("gvl", "lhsq", "pow8", "ci16", "qt3", "p0rT")
    names = [n for n in first if n in IN_SPECS] + \
            [n for n in IN_SPECS if n not in first + ("rhsd2", "gvrhs")]
    for name in names:
        shape, dt = IN_SPECS[name]
        t = consts.tile(list(shape), dt, tag=f"c_{name}")
        nc.sync.dma_start(out=t[:], in_=ins[name])
        ct[name] = t
        if name == "gvl":
            load_quarter(0, split=2)
        elif name == "pow8":
            load_quarter(1)
    cf = ct['cf32']
    ci = ct['ci16']

    c999 = consts.tile([128, SEL2], I16, tag="c999")
    nc.vector.memset(c999[:], 999)
    ones32 = consts.tile([128, NSLOT], I16, tag="ones32")
    nc.vector.memset(ones32[:], 1)

    qdB = gvp.tile([128, 512], F32, tag="qdB")
    idxall = gvp.tile([128, 512], F32, tag="idxall")
    ceffall = gvp.tile([128, NQT], F32, tag="ceffall")
    outbuf = outp.tile([128, NQ], F32, tag="outbuf")

    # ---- prologue: query deltas ----
    psQ = psA.tile([128, 512], F32, tag="psa")
    for uq in range(4):
        rhs = ct['qt3'][:].rearrange("c (i uu q) -> c uu i q", uu=4, q=32)[:, uq]
        nc.tensor.matmul(psQ[32 * uq:32 * uq + 32, :],
                         ct['p0rT'][:, 32 * uq:32 * uq + 32],
                         rhs, start=True, stop=True,
                         tile_position=(0, 32 * uq))
    nc.vector.tensor_scalar(qdB[:], psQ[:], cf[:, 400:401], None, ALU.add)

    # ==========================================================
    def build_table(i):
        """Gtab_i f32 = [G window | V window] for tile i."""
        gvrT = gvrTq[(i // 4) % 2]
        lo = NT2 * (i % 4)                # offset within the quarter buffer
        tab = tabp.tile([128, 2 * NT2], F32, tag="tab")
        for half, lhs in ((0, 0), (1, 1)):
            gvl_lhs = ct['gvl'][:, 128 * lhs:128 * lhs + 128]
            for off, w in CHUNKS:
                p = psA.tile([128, 512], F32, tag="psa")
                nc.tensor.matmul(p[:, 0:w], gvl_lhs,
                                 gvrT[:, lo + off:lo + off + w],
                                 start=True, stop=True)
                nc.scalar.activation(
                    tab[:, NT2 * half + off:NT2 * half + off + w],
                    p[:, 0:w], ACTF.Copy)
        return tab

    # ==========================================================
    def phase_a(i):
        rhsT = rhsTq[(i // 4) % 2]
        lo = NT2 * (i % 4)
        vw8c = selbig.tile([128, NT2], BF16, tag="vw8c")
        for off, w in CHUNKS:
            pd2 = psA.tile([128, 512], F32, tag="psa")
            nc.tensor.matmul(pd2[:, 0:w], ct['lhsq'][:, bass.ts(i, 128)],
                             rhsT[:, lo + off:lo + off + w],
                             start=True, stop=True)
            nc.vector.scalar_tensor_tensor(
                vw8c[:, off:off + w], pd2[:, 0:w], R2, ct['pow8'][:, 0:w],
                op0=ALU.is_lt, op1=ALU.mult)

        w8i = sel.tile([128, NG], I16, tag="w8i")
        nc.vector.tensor_reduce(
            w8i[:], vw8c[:].rearrange("p (w t) -> p w t", t=8),
            mybir.AxisListType.X, ALU.add)

        nz = small.tile([128, NG], I16, tag="nz")
        nc.vector.tensor_scalar(nz[:], w8i[:], 0.0, None, ALU.is_gt)
        crank = small.tile([128, NG], I16, tag="crank")
        nc.vector.tensor_tensor_scan(crank[:], nz[:], c999[:, 0:NG], 0.0,
                                     ALU.add, ALU.min)
        u = small.tile([128, NG], I16, tag="u")
        nc.vector.tensor_tensor(u[:], crank[:], nz[:], ALU.mult)
        si16 = small.tile([128, NG], I16, tag="si16")
        nc.vector.tensor_scalar(si16[:], u[:], -1.0, None, ALU.add)

        dstID = small.tile([128, NG], I16, tag="dstID")
        dstW = small.tile([128, NG], I16, tag="dstW")
        nc.gpsimd.local_scatter(dstID[:], ci[:, 0:NG], si16[:], 128, NG, NG)
        nc.gpsimd.local_scatter(dstW[:], w8i[:], si16[:], 128, NG, NG)

        esel16 = small.tile([128, SEL2], I16, tag="esel16")
        evb = esel16[:].rearrange("p (s t) -> p s t", t=8)
        for t in range(8):
            nc.vector.scalar_tensor_tensor(
                evb[:, :, t], dstW[:, 0:NSLOT], ci[:, NG + t:NG + t + 1], ones32[:],
                op0=ALU.logical_shift_right, op1=ALU.bitwise_and)

        crank2 = small.tile([128, SEL2], I16, tag="crank2")
        nc.vector.tensor_tensor_scan(crank2[:], esel16[:], c999[:], 0.0,
                                     ALU.add, ALU.min)
        u2 = small.tile([128, SEL2], I16, tag="u2")
        nc.vector.tensor_tensor(u2[:], crank2[:], esel16[:], ALU.mult)
        si2 = small.tile([128, SEL2], I16, tag="si2")
        nc.vector.tensor_scalar(si2[:], u2[:], -1.0, None, ALU.add)

        tmp8 = small.tile([128, NSLOT], I16, tag="tmp8")
        nc.vector.tensor_scalar(tmp8[:], dstID[:, 0:NSLOT], 8.0, None, ALU.mult)
        cjp1 = small.tile([128, SEL2], I16, tag="cjp1")
        nc.vector.tensor_tensor(
            cjp1[:].rearrange("p (s t) -> p s t", t=8),
            tmp8[:].unsqueeze(2).broadcast_to((128, NSLOT, 8)),
            ci[:, NG + 8:NG + 8 + SEL2].rearrange("p (s t) -> p s t", t=8), ALU.add)
        idxp1 = small.tile([128, SEL2], I16, tag="idxp1")
        nc.gpsimd.local_scatter(idxp1[:], cjp1[:], si2[:], 128, SEL2, SEL2)

        # per-tile fill -> idxall[:, 32i:32i+32]
        ii = small.tile([128, 32], F32, tag="ii")
        nc.vector.tensor_scalar(ii[:], idxp1[:, 0:32], 0.0, None, ALU.add)
        flp1 = small.tile([128, 1], F32, tag="flp1")
        nc.vector.tensor_scalar(flp1[:], ii[:, 0:1], 1.0, None, ALU.max)
        m = small.tile([128, 32], F32, tag="m")
        nc.vector.tensor_scalar(m[:], ii[:], 0.0, None, ALU.is_gt)
        bb = small.tile([128, 32], F32, tag="bb")
        nc.vector.tensor_tensor(bb[:], ii[:], m[:], ALU.mult)
        aa = small.tile([128, 32], F32, tag="aa")
        nc.vector.tensor_scalar(aa[:], m[:], flp1[:], None, ALU.mult)
        cc = small.tile([128, 32], F32, tag="cc")
        nc.vector.tensor_tensor(cc[:], bb[:], aa[:], ALU.subtract)
        nc.vector.tensor_scalar(idxall[:, bass.ts(i, 32)], cc[:], flp1[:], -1.0,
                                ALU.add, ALU.add)

        cnt0 = small.tile([128, 1], F32, tag="cnt0")
        nc.vector.tensor_scalar(cnt0[:], crank2[:, SEL2 - 1:SEL2], 32.0, None,
                                ALU.min)
        qfc = small.tile([128, 1], F32, tag="qfc")
        nc.vector.tensor_scalar(qfc[:], cf[:, 384 + i:385 + i], -32.0, 32.0,
                                ALU.mult, ALU.add)
        nc.vector.tensor_tensor(ceffall[:, i:i + 1], cnt0[:], qfc[:], ALU.max)


    # ==========================================================
    def block_bg(blk):
        """beta/gamma rows for the 4 tiles of blk -> bgrow8 [4, 256] f32r."""
        ceff4 = ceffall[:, blk * GBLK:(blk + 1) * GBLK]
        beta4 = small.tile([128, GBLK], F32, tag="beta4")
        nc.vector.reciprocal(beta4[:], ceff4)
        gm4 = small.tile([128, GBLK], F32, tag="gm4")
        nc.vector.tensor_scalar(gm4[:], ceff4, -1.0, 32.0, ALU.mult, ALU.add)
        gamma4 = small.tile([128, GBLK], F32, tag="gamma4")
        nc.vector.tensor_tensor(gamma4[:], gm4[:], beta4[:], ALU.mult)
        psT2 = psA.tile([GBLK, 256], F32, tag="psa")
        nc.tensor.matmul(psT2[:, 0:128], beta4[:], cf[:, 256:384],
                         start=True, stop=True)
        nc.tensor.matmul(psT2[:, 128:256], gamma4[:], cf[:, 256:384],
                         start=True, stop=True)
        bgrow8 = small.tile([GBLK, 256], F32R, tag="bgrow8")
        nc.scalar.activation(bgrow8[:], psT2[:], ACTF.Copy)
        return bgrow8

    # ==========================================================
    def idx_gather(i, tab):
        psW = psA.tile([128, 64], F32, tag="psa")
        nc.tensor.matmul(psW[:, 0:32], cf[:, 0:128], idxall[:, bass.ts(i, 32)],
                         start=True, stop=True)
        nc.tensor.matmul(psW[:, 32:64], cf[:, 128:256], idxall[:, bass.ts(i, 32)],
                         start=True, stop=True)
        idxB = idxp.tile([128, NIDX // 16], I16, tag="idxB")
        nc.scalar.activation(
            idxB[:, 0:64].rearrange("p (r two) -> p two r", two=2),
            psW[:, 0:64].rearrange("p (two r) -> p two r", two=2), ACTF.Copy)
        vv = psW[:, 0:64].rearrange("p (h r) -> p h r", h=2)[:, :, 0]
        nc.vector.tensor_scalar(idxB[:, 64:66], vv, float(NT2), None, ALU.add)
        gout = gpool.tile([128, NIDX], F32, tag="gout")
        nc.gpsimd.ap_gather(gout[:].unsqueeze(2), tab[:].unsqueeze(2),
                            idxB[:], 128, 2 * NT2, 1, NIDX)
        return gout

    # ==========================================================
    def phase_b(i, gout):
        gv = gout[:, 0:1024]
        v8 = gout[:, 1024:1056]

        ceff = ceffall[:, i:i + 1]
        beta = small.tile([128, 1], F32, tag="beta")
        nc.vector.reciprocal(beta[:], ceff)
        gm0 = small.tile([128, 1], F32, tag="gm0")
        nc.vector.tensor_scalar(gm0[:], ceff, -1.0, 32.0, ALU.mult, ALU.add)
        gamma = small.tile([128, 1], F32, tag="gamma")
        nc.vector.tensor_tensor(gamma[:], gm0[:], beta[:], ALU.mult)
        psBG = psA.tile([1, 256], F32, tag="psa")
        nc.tensor.matmul(psBG[:, 0:128], beta[:], cf[:, 256:384],
                         start=True, stop=True)
        nc.tensor.matmul(psBG[:, 128:256], gamma[:], cf[:, 256:384],
                         start=True, stop=True)
        bgrow = small.tile([1, 256], F32, tag="bgrow")
        if i % 2 == 0:
            nc.scalar.activation(bgrow[:], psBG[:], ACTF.Copy)
        else:
            nc.vector.tensor_copy(bgrow[:], psBG[:])
        psB = psA.tile([128, 256], F32, tag="psa")
        nc.tensor.matmul(psB[:], ct['onesk1'][:], bgrow[:], start=True, stop=True)
        dd = small.tile([128, 32], F32, tag="dd")
        nc.vector.tensor_tensor(dd[:], v8, qdB[:, bass.ts(i, 32)], ALU.add)
        h1t = mlpp.tile([128, 1024], F32, tag="h1t")
        nc.vector.tensor_tensor(
            h1t[:].rearrange("p (r q) -> p r q", q=32),
            gv.rearrange("p (r q) -> p r q", q=32),
            dd[:].unsqueeze(1).broadcast_to((128, 32, 32)), ALU.add)
        h1 = mlpp.tile([128, 1024], F32R, tag="h1")
        nc.scalar.activation(h1[:], h1t[:], ACTF.Relu)

        pL2 = psL2.tile([128, 1024], F32, tag="psl2")
        for n in range(2):
            nc.tensor.matmul(pL2[:, bass.ts(n, 512)], ct['w1blk'][:],
                             h1[:, bass.ts(n, 512)], start=True, stop=True)
        h2 = mlpp.tile([128, 1024], F32R, tag="h2")
        nc.scalar.activation(h2[:], pL2[:], ACTF.Relu, bias=cf[:, 401:402])

        h3 = h3p.tile([128, 4096], BF16, tag="h3")
        for g in range(4):
            pL3 = psL3.tile([128, 1024], F32, tag="psl3")
            for n in range(2):
                nc.tensor.matmul(pL3[:, bass.ts(n, 512)],
                                 ct['w2t4'][32 * g:32 * g + 32, :],
                                 h2[32 * g:32 * g + 32, bass.ts(n, 512)],
                                 start=True, stop=True,
                                 tile_position=(32 * g, 0))
            nc.scalar.activation(h3[:, bass.ts(g, 1024)], pL3[:], ACTF.Relu,
                                 bias=cf[:, 402:403])

        # bf16 tree pooling over r (h3 cols are (g r q))
        ha = mlpp.tile([128, 2048], BF16, tag="ha")
        va = h3[:].rearrange("p (g r two q) -> p g r two q", g=4, two=2, q=32)
        nc.vector.tensor_tensor(ha[:].rearrange("p (g r q) -> p g r q", g=4, q=32),
                                va[:, :, :, 0], va[:, :, :, 1], ALU.add)
        hb = mlpp.tile([128, 1024], BF16, tag="hb")
        vb = ha[:].rearrange("p (g r two q) -> p g r two q", g=4, two=2, q=32)
        nc.vector.tensor_tensor(hb[:].rearrange("p (g r q) -> p g r q", g=4, q=32),
                                vb[:, :, :, 0], vb[:, :, :, 1], ALU.add)
        hc = mlpp.tile([128, 512], BF16, tag="hc")
        vc = hb[:].rearrange("p (g r two q) -> p g r two q", g=4, two=2, q=32)
        nc.vector.tensor_tensor(hc[:].rearrange("p (g r q) -> p g r q", g=4, q=32),
                                vc[:, :, :, 0], vc[:, :, :, 1], ALU.add)
        hd = mlpp.tile([128, 256], BF16, tag="hd")
        vd = hc[:].rearrange("p (g r two q) -> p g r two q", g=4, two=2, q=32)
        nc.vector.tensor_tensor(hd[:].rearrange("p (g r q) -> p g r q", g=4, q=32),
                                vd[:, :, :, 0], vd[:, :, :, 1], ALU.add)
        S = small.tile([128, 128], F32, tag="S")
        ve = hd[:].rearrange("p (g two q) -> p g two q", g=4, q=32)
        nc.vector.tensor_tensor(S[:].rearrange("p (g q) -> p g q", g=4),
                                ve[:, :, 0], ve[:, :, 1], ALU.add)

        h30 = small.tile([128, 128], F32, tag="h30")
        nc.scalar.activation(
            h30[:].rearrange("p (g q) -> p g q", g=4),
            h3[:].rearrange("p (g r q) -> p g r q", g=4, q=32)[:, :, 0, :],
            ACTF.Copy)


        e1 = small.tile([128, 128], F32, tag="e1")
        nc.vector.tensor_tensor(e1[:], S[:], psB[:, 0:128], ALU.mult)
        e2 = small.tile([128, 128], F32, tag="e2")
        nc.vector.tensor_tensor(e2[:], h30[:], psB[:, 128:256], ALU.mult)
        nc.vector.tensor_tensor(outbuf[:, bass.ts(i, 128)], e1[:], e2[:],
                                ALU.subtract)

    # ==== emission: software pipeline, B(blk-1) interleaved into blk ====
    prev_gouts = None
    pre_tab = None
    for blk in range(NBLK):
        if blk in (1, 2):
            load_quarter(blk + 1)
        tabs = []
        gnew = []
        for il in range(GBLK):
            if il == 0 and pre_tab is not None:
                tabs.append(pre_tab)
            else:
                tabs.append(build_table(blk * GBLK + il))
            phase_a(blk * GBLK + il)
            if prev_gouts is not None:
                phase_b((blk - 1) * GBLK + il, prev_gouts[il])
            if il >= 1:
                gnew.append(idx_gather(blk * GBLK + il - 1, tabs[il - 1]))
        if blk < NBLK - 1:     # quarter-staged loads make this always legal
            pre_tab = build_table((blk + 1) * GBLK)
        else:
            pre_tab = None
        gnew.append(idx_gather(blk * GBLK + GBLK - 1, tabs[GBLK - 1]))
        prev_gouts = gnew
        if blk > 0:
            nc.sync.dma_start(out=out_ap[:, bass.ts(blk - 1, 512)],
                              in_=outbuf[:, bass.ts(blk - 1, 512)])
    for il in range(GBLK):
        i = (NBLK - 1) * GBLK + il
        phase_b(i, prev_gouts[il])
        nc.sync.dma_start(out=out_ap[:, bass.ts(i, 128)],
                          in_=outbuf[:, bass.ts(i, 128)])


# ==========================================================================
_CACHE = {}


def _build_nc():
    import concourse.bacc as bacc
    import concourse.tile as tile_mod
    nc = bacc.Bacc("TRN2", target_bir_lowering=False, debug=False, num_devices=8)
    in_tiles = {}
    for name, (shape, dt) in IN_SPECS.items():
        in_tiles[name] = nc.dram_tensor(
            name, list(shape), dt, kind="ExternalInput").ap()
    out_tile = nc.dram_tensor("out", (128, NQ), F32, kind="ExternalOutput").ap()
    with tile_mod.TileContext(nc) as t:
        build_kernel(t, out_tile, in_tiles)
    nc.compile()
    return nc


def kernel(**inputs):
    from concourse.bass_utils import run_bass_kernel_spmd
    in_maps, meta = host_prep(inputs)
    if "nc" not in _CACHE:
        _CACHE["nc"] = _build_nc()
    res = run_bass_kernel_spmd(_CACHE["nc"], in_maps, list(range(8)))
    return host_finish(res.results, meta)

